# revision 21
# baseline (speedup 1.0000x reference)
"""Trainium2 Bass kernel for nn_DecoderRNN_Attn (attention decoder LSTM).

Problem shapes: B=32, T=32 (31 decode steps), S=64, H=E=512, V=32000.

Strategy (8 NeuronCores):
  - The LSTM+attention recurrence (sequential over t) is replicated on all
    cores in fp32.  Per step, activations are kept both in normal [B, feat]
    orientation and transposed [feat, B] orientation (via PE transposes) so
    every matmul can use the activation as the stationary operand and stream
    the large weight operand at full rate.
  - Attention uses a transposed-scores formulation:
        scoresT[(b,s), b'] = projT.T @ h2T        (projT stationary)
        alignT = exp(scoresT) * maskT             (mask after exp; scores are
                                                   O(1) so no overflow)
        denom[1, b] = ones.T @ alignT             (off-diagonal entries are 0)
        ctx[b, h]   = alignT.T @ enc_flat         (alignT already stationary)
    which needs no per-step transposes of the attention matrix.
  - The embedding contribution X @ W_x^T + (b_ih + b_hh) for all 31 steps is
    precomputed on-device as one batched matmul (it does not depend on the
    recurrence), stored in DRAM, and injected into the per-step gate PSUM via
    an identity matmul.
  - The output projection is tensor-parallel over the vocabulary: each core
    keeps a [512, 4000] shard of W_o^T in SBUF and computes logits for all
    992 = 31*32 (t, b) rows at full PE utilisation.  log_softmax needs the
    global (all-vocab) sum of exp(logit); each core computes its partial sums
    and one 4 KB AllReduce combines them, after which each core writes its
    normalized log-prob shard.  Argmax candidates (top-8 per row per half-
    tile, in the exp domain, fp32) are written out and combined on the host.

Host-side work is restricted to dtype casts, transposes / reshapes / slicing
of inputs (layout marshalling), the embedding row gather, sharding W_o/b_o,
and the final unshard (concat of log-prob shards + argmax-candidate combine).
"""

import os
from contextlib import ExitStack

import numpy as np

B, T, S, H, E, V = 32, 32, 64, 512, 512, 32000
NCORES = 8
VS = V // NCORES          # 4000 vocab shard per core
TT = T - 1                # 31 decode steps
R = TT * B                # 992 output rows
MT = 8                    # output-row tiles
MP = R // MT              # 124 rows per tile
NHALF = VS // 2           # 2000 (half of the vocab shard, 4 PSUM banks)
KH = H // 128             # 4 k-tiles for one H contraction

_f32 = None  # set lazily (mybir import)


def _build(nc, tc, ctx, tt=TT):
    import concourse.bass as bass
    import concourse.mybir as mybir
    from concourse.masks import make_identity

    R_ = tt * B
    MT_ = max(1, R_ // MP) if R_ % MP == 0 else (R_ + MP - 1) // MP
    MP_ = R_ // MT_
    assert MT_ * MP_ == R_

    f32 = mybir.dt.float32
    bf16 = mybir.dt.bfloat16
    i32 = mybir.dt.int32
    u32 = mybir.dt.uint32
    AF = mybir.ActivationFunctionType
    ALU = mybir.AluOpType
    AX = mybir.AxisListType

    # ---------------- DRAM I/O ----------------
    xt = nc.dram_tensor("xt", [H, R_], f32, kind="ExternalInput")          # X^T
    w_xt = nc.dram_tensor("w_xt", [E, 4 * H], f32, kind="ExternalInput")  # W_x^T
    w_combt = nc.dram_tensor("w_combt", [2 * H, 4 * H], f32, kind="ExternalInput")
    w_ct = nc.dram_tensor("w_ct", [2 * H, H], f32, kind="ExternalInput")
    w_at = nc.dram_tensor("w_at", [H, H], f32, kind="ExternalInput")      # W_a^T
    enc_flat = nc.dram_tensor("enc_flat", [B * S, H], f32, kind="ExternalInput")
    enct = nc.dram_tensor("enct", [H, B * S], f32, kind="ExternalInput")
    w_ot = nc.dram_tensor("w_ot", [H, VS], f32, kind="ExternalInput")     # shard
    b_ih = nc.dram_tensor("b_ih", [1, 4 * H], f32, kind="ExternalInput")
    b_hh = nc.dram_tensor("b_hh", [1, 4 * H], f32, kind="ExternalInput")
    b_c = nc.dram_tensor("b_c", [1, H], f32, kind="ExternalInput")
    b_o = nc.dram_tensor("b_o", [1, VS], f32, kind="ExternalInput")       # shard
    len_f = nc.dram_tensor("len_f", [B, 1], f32, kind="ExternalInput")
    h0 = nc.dram_tensor("h0", [B, H], f32, kind="ExternalInput")
    c0 = nc.dram_tensor("c0", [B, H], f32, kind="ExternalInput")

    logp = nc.dram_tensor("logp", [R_, VS], f32, kind="ExternalOutput")
    maxv = nc.dram_tensor("maxv", [MT_, 2, MP_, 8], f32, kind="ExternalOutput")
    maxi = nc.dram_tensor("maxi", [MT_, 2, MP_, 8], u32, kind="ExternalOutput")

    xw_dram = nc.dram_tensor("xw_scratch", [R_, 4 * H], f32)               # internal
    cc_in = nc.dram_tensor("cc_in", [128, MT_], f32)
    cc_out = nc.dram_tensor("cc_out", [128, MT_], f32, addr_space="Shared")

    # ---------------- persistent SBUF ----------------
    singles = ctx.enter_context(tc.tile_pool(name="singles", bufs=1))

    ident = singles.tile([128, 128], f32)
    make_identity(nc, ident)
    ones_col = singles.tile([128, 1], f32)      # ones column (denominator mm)
    nc.vector.memset(ones_col, 1.0)
    ones_row = singles.tile([1, 128], f32)      # ones row (bias-inject mms)
    nc.vector.memset(ones_row, 1.0)

    maskt_sb = singles.tile([128, B * S // 128, B], f32)    # 0/1 mask, transposed
    aht_all = singles.tile([128, KH, R_], f32)            # 2 MB, lhsT for phase 2
    bc_sb = singles.tile([1, H], f32)
    len_sb = singles.tile([B, 1], f32)
    nc.sync.dma_start(out=len_sb, in_=len_f.ap())

    # ================= PREP PHASE =================
    with tc.tile_pool(name="prep", bufs=1) as prep:
        # ---- mask (normal orientation), then transpose once ----
        # mask_norm[b', col=(b*64+s)] = 1 if (b'==b and s < len[b']) else 0
        vi = prep.tile([B, B * S], i32)
        nc.gpsimd.iota(vi, pattern=[[1, B * S]], base=0, channel_multiplier=-S)
        vf = prep.tile([B, B * S], f32)
        nc.vector.tensor_copy(out=vf, in_=vi)
        m1 = prep.tile([B, B * S], f32)
        nc.vector.tensor_scalar(out=m1, in0=vf, scalar1=0.0, scalar2=None,
                                op0=ALU.is_ge)
        m2 = prep.tile([B, B * S], f32)
        nc.vector.tensor_scalar(out=m2, in0=vf, scalar1=len_sb, scalar2=None,
                                op0=ALU.is_lt)
        mask_norm = prep.tile([B, B * S], f32)
        nc.vector.tensor_mul(out=mask_norm, in0=m1, in1=m2)
        with tc.tile_pool(name="ps_prep_t", bufs=2, space="PSUM") as pst:
            for kt in range(B * S // 128):
                tp = pst.tile([128, B], f32, tag="tr")
                nc.tensor.transpose(tp, mask_norm[:, kt * 128:(kt + 1) * 128],
                                    ident[:B, :B])
                nc.vector.tensor_copy(out=maskt_sb[:, kt, :], in_=tp)

        # ---- XW = X^T.T @ [W_x^T; bias] for all steps, -> DRAM ----
        xt_sb = prep.tile([128, KH, R_], f32, tag="xt")
        for c in range(KH):
            nc.sync.dma_start(out=xt_sb[:, c, :],
                              in_=xt.ap()[c * 128:(c + 1) * 128, :])
        w_xt_sb = prep.tile([128, KH, 4 * H], f32, tag="wxt")
        for c in range(KH):
            nc.sync.dma_start(out=w_xt_sb[:, c, :],
                              in_=w_xt.ap()[c * 128:(c + 1) * 128, :])
        bih_sb = prep.tile([1, 4 * H], f32, tag="bih")
        nc.sync.dma_start(out=bih_sb, in_=b_ih.ap())
        bhh_sb = prep.tile([1, 4 * H], f32, tag="bhh")
        nc.sync.dma_start(out=bhh_sb, in_=b_hh.ap())
        bcomb_sb = prep.tile([1, 4 * H], f32, tag="bcomb")
        nc.vector.tensor_add(out=bcomb_sb, in0=bih_sb, in1=bhh_sb)

        with tc.tile_pool(name="ps_prep", bufs=2, space="PSUM") as psp:
            for mt in range(MT_):
                ms = slice(mt * MP_, (mt + 1) * MP_)
                xwp = psp.tile([MP_, 4 * H], f32, tag="xw")
                for nch in range(4):
                    ns = slice(nch * 512, (nch + 1) * 512)
                    nc.tensor.matmul(xwp[:, ns], ones_row[:, :MP_],
                                     bcomb_sb[:, ns], start=True, stop=False)
                    for kc in range(KH):
                        nc.tensor.matmul(xwp[:, ns], xt_sb[:, kc, ms],
                                         w_xt_sb[:, kc, ns], start=False,
                                         stop=(kc == KH - 1))
                xws = prep.tile([MP_, 4 * H], f32, tag="xws")
                nc.vector.tensor_copy(out=xws, in_=xwp)
                nc.sync.dma_start(out=xw_dram[ms, :], in_=xws)

    # ---- recurrence-lifetime SBUF (freed before phase 2) ----
    ph1 = ExitStack()
    recur = ph1.enter_context(tc.tile_pool(name="recur", bufs=1))
    projt_sb = recur.tile([128, KH, B * S], f32)         # 4.2 MB
    encflat_sb = recur.tile([128, B * S // 128, H], f32)  # 4.2 MB
    w_combt_sb = recur.tile([128, 8, 4 * H], f32)        # 8.4 MB
    w_ct_sb = recur.tile([128, 8, H], f32)               # 2.1 MB

    # ---- projT = (W_a^T).T @ encT  (4 m-tiles over k) ----
    with tc.tile_pool(name="prep2", bufs=1) as prep2:
        enct_sb = prep2.tile([128, KH, B * S], f32, tag="enct")
        for c in range(KH):
            nc.sync.dma_start(out=enct_sb[:, c, :],
                              in_=enct.ap()[c * 128:(c + 1) * 128, :])
        w_at_sb = prep2.tile([128, KH, H], f32, tag="wat")
        for c in range(KH):
            nc.sync.dma_start(out=w_at_sb[:, c, :],
                              in_=w_at.ap()[c * 128:(c + 1) * 128, :])
        for c in range(B * S // 128):
            nc.sync.dma_start(out=encflat_sb[:, c, :],
                              in_=enc_flat.ap()[c * 128:(c + 1) * 128, :])
        nc.sync.dma_start(out=bc_sb, in_=b_c.ap())
        with tc.tile_pool(name="ps_prep2", bufs=2, space="PSUM") as psp2:
            for kt in range(KH):   # output k-tile (partition dim of projT)
                pp = psp2.tile([128, B * S], f32, tag="proj")
                for nch in range(B * S // 512):
                    ns = slice(nch * 512, (nch + 1) * 512)
                    for hc in range(KH):
                        nc.tensor.matmul(pp[:, ns],
                                         w_at_sb[:, hc, kt * 128:(kt + 1) * 128],
                                         enct_sb[:, hc, ns], start=(hc == 0),
                                         stop=(hc == KH - 1))
                nc.vector.tensor_copy(out=projt_sb[:, kt, :], in_=pp)

    # load the big recurrent weights
    for c in range(8):
        nc.sync.dma_start(out=w_combt_sb[:, c, :],
                          in_=w_combt.ap()[c * 128:(c + 1) * 128, :])
    for c in range(8):
        nc.sync.dma_start(out=w_ct_sb[:, c, :],
                          in_=w_ct.ap()[c * 128:(c + 1) * 128, :])

    # ================= RECURRENCE =================
    state = ph1.enter_context(tc.tile_pool(name="state", bufs=2))
    work = ph1.enter_context(tc.tile_pool(name="work", bufs=1))
    xwpool = ph1.enter_context(tc.tile_pool(name="xwslice", bufs=2))

    h_sb = state.tile([B, H], f32, tag="h")
    nc.sync.dma_start(out=h_sb, in_=h0.ap())
    c_sb = state.tile([B, H], f32, tag="c")
    nc.sync.dma_start(out=c_sb, in_=c0.ap())

    psA = ph1.enter_context(tc.tile_pool(name="ps_big", bufs=1, space="PSUM"))
    psB = ph1.enter_context(tc.tile_pool(name="ps_tr", bufs=2, space="PSUM"))
    psC = ph1.enter_context(tc.tile_pool(name="ps_misc", bufs=1, space="PSUM"))

    # initial transposed states: hT from h0, ahT = 0
    inT_sb = state.tile([128, 8, B], f32, tag="inT")   # [ahT(4) ; hT(4)]
    nc.vector.memset(inT_sb[:, :KH, :], 0.0)
    for c in range(KH):
        tp = psB.tile([128, B], f32, tag="tr")
        nc.tensor.transpose(tp, h_sb[:, c * 128:(c + 1) * 128], ident[:B, :B])
        nc.vector.tensor_copy(out=inT_sb[:, KH + c, :], in_=tp)

    for t in range(tt):
        xw_sb = xwpool.tile([128, 512], f32, tag="xw")
        for cch in range(4):
            nc.sync.dma_start(
                out=xw_sb[cch * B:(cch + 1) * B, :],
                in_=xw_dram[t * B:(t + 1) * B, cch * 512:(cch + 1) * 512])

        # ---- gates = XW[t] + [ah; h] @ W_comb^T ----
        gates_ps = psA.tile([B, 4 * H], f32, tag="big")
        for nch in range(4):
            ns = slice(nch * 512, (nch + 1) * 512)
            nc.tensor.matmul(gates_ps[:, ns],
                             ident[nch * B:(nch + 1) * B, nch * B:(nch + 1) * B],
                             xw_sb[nch * B:(nch + 1) * B, :],
                             start=True, stop=False,
                             tile_position=(nch * B, 0))
            for kc in range(8):
                nc.tensor.matmul(gates_ps[:, ns], inT_sb[:, kc, :],
                                 w_combt_sb[:, kc, ns], start=False,
                                 stop=(kc == 7))

        # ---- LSTM pointwise (torch gate order i, f, g, o) ----
        i_sb = work.tile([B, H], f32, tag="i")
        f_sb = work.tile([B, H], f32, tag="f")
        g_sb = work.tile([B, H], f32, tag="g")
        o_sb = work.tile([B, H], f32, tag="o")
        nc.scalar.activation(out=i_sb, in_=gates_ps[:, 0:H], func=AF.Sigmoid)
        nc.scalar.activation(out=f_sb, in_=gates_ps[:, H:2 * H], func=AF.Sigmoid)
        nc.scalar.activation(out=g_sb, in_=gates_ps[:, 2 * H:3 * H], func=AF.Tanh)
        nc.scalar.activation(out=o_sb, in_=gates_ps[:, 3 * H:4 * H], func=AF.Sigmoid)
        ig = work.tile([B, H], f32, tag="ig")
        nc.vector.tensor_mul(out=ig, in0=i_sb, in1=g_sb)
        fc = work.tile([B, H], f32, tag="fc")
        nc.vector.tensor_mul(out=fc, in0=f_sb, in1=c_sb)
        c_new = state.tile([B, H], f32, tag="c")
        nc.vector.tensor_add(out=c_new, in0=ig, in1=fc)
        c_sb = c_new
        tc2 = work.tile([B, H], f32, tag="tc2")
        nc.scalar.activation(out=tc2, in_=c_sb, func=AF.Tanh)
        h2_sb = state.tile([B, H], f32, tag="h")
        nc.vector.tensor_mul(out=h2_sb, in0=o_sb, in1=tc2)

        # ---- transposed h2 -> next inT (h part) + scores rhs ----
        inT_new = state.tile([128, 8, B], f32, tag="inT")
        for c in range(KH):
            tp = psB.tile([128, B], f32, tag="tr")
            nc.tensor.transpose(tp, h2_sb[:, c * 128:(c + 1) * 128],
                                ident[:B, :B])
            nc.vector.tensor_copy(out=inT_new[:, KH + c, :], in_=tp)

        # ---- scoresT[(b,s), b'] (16 m-tiles), exp, mask ----
        sct_ps = psC.tile([128, B * S // 128, B], f32, tag="sct")
        for mt16 in range(B * S // 128):
            for kc in range(KH):
                nc.tensor.matmul(sct_ps[:, mt16, :],
                                 projt_sb[:, kc, mt16 * 128:(mt16 + 1) * 128],
                                 inT_new[:, KH + kc, :], start=(kc == 0),
                                 stop=(kc == KH - 1))
        alignt_sb = work.tile([128, B * S // 128, B], f32, tag="alignt")
        nc.scalar.activation(out=alignt_sb, in_=sct_ps, func=AF.Exp)
        nc.vector.tensor_mul(out=alignt_sb, in0=alignt_sb, in1=maskt_sb)

        # ---- denominator (col sums; off-diagonal entries are zero) ----
        den_ps = psB.tile([1, B], f32, tag="tr", name="den_ps")
        for kt in range(B * S // 128):
            nc.tensor.matmul(den_ps, ones_col, alignt_sb[:, kt, :],
                             start=(kt == 0), stop=(kt == B * S // 128 - 1))
        den_sb = work.tile([1, B], f32, tag="den_sb")
        nc.vector.tensor_copy(out=den_sb, in_=den_ps)
        denT_ps = psB.tile([B, 1], f32, tag="tr", name="denT_ps")
        nc.tensor.transpose(denT_ps, den_sb, ident[:1, :1])
        rec_sb = work.tile([B, 1], f32, tag="rec")
        nc.vector.reciprocal(out=rec_sb, in_=denT_ps)

        # ---- ctx = alignT.T @ enc_flat, normalized by 1/denom ----
        ctx_ps = psC.tile([B, H], f32, tag="cw", name="ctx_ps")
        for kt in range(B * S // 128):
            nc.tensor.matmul(ctx_ps, alignt_sb[:, kt, :], encflat_sb[:, kt, :],
                             start=(kt == 0), stop=(kt == B * S // 128 - 1))
        ctx_sb = work.tile([B, H], f32, tag="ctx_sb")
        nc.vector.tensor_scalar(out=ctx_sb, in0=ctx_ps, scalar1=rec_sb,
                                scalar2=None, op0=ALU.mult)

        # ---- wcT input = [h2T ; ctxT] ----
        wct_in = work.tile([128, 8, B], f32, tag="wct_in")
        nc.vector.tensor_copy(out=wct_in[:, :KH, :], in_=inT_new[:, KH:, :])
        for c in range(KH):
            tp = psB.tile([128, B], f32, tag="tr")
            nc.tensor.transpose(tp, ctx_sb[:, c * 128:(c + 1) * 128],
                                ident[:B, :B])
            nc.vector.tensor_copy(out=wct_in[:, KH + c, :], in_=tp)

        # ---- ah2 = tanh([h2, ctx] @ W_c^T + b_c) ----
        wc_ps = psC.tile([B, H], f32, tag="cw", name="wc_ps")
        nc.tensor.matmul(wc_ps, ones_row[:, :B], bc_sb, start=True, stop=False)
        for kc in range(8):
            nc.tensor.matmul(wc_ps, wct_in[:, kc, :], w_ct_sb[:, kc, :],
                             start=False, stop=(kc == 7))
        ah2_sb = state.tile([B, H], f32, tag="ah")
        nc.scalar.activation(out=ah2_sb, in_=wc_ps, func=AF.Tanh)

        # ---- ah2T -> inT (ah part) + AHT accumulation ----
        for c in range(KH):
            tp = psB.tile([128, B], f32, tag="tr")
            nc.tensor.transpose(tp, ah2_sb[:, c * 128:(c + 1) * 128],
                                ident[:B, :B])
            nc.vector.tensor_copy(out=inT_new[:, c, :], in_=tp)
        nc.vector.tensor_copy(out=aht_all[:, :, t * B:(t + 1) * B],
                              in_=inT_new[:, :KH, :])
        inT_sb = inT_new

    ph1.close()  # frees recurrence-lifetime SBUF/PSUM before phase 2

    # ================= PHASE 2: vocab projection =================
    ph2 = ctx.enter_context(tc.tile_pool(name="ph2", bufs=1))
    w_ot_sb = ph2.tile([128, KH, VS], f32)
    for c in range(KH):
        nc.sync.dma_start(out=w_ot_sb[:, c, :],
                          in_=w_ot.ap()[c * 128:(c + 1) * 128, :])
    bo_sb = ph2.tile([1, VS], f32)
    nc.sync.dma_start(out=bo_sb, in_=b_o.ap())
    logits_sb = ph2.tile([128, MT_, VS], bf16)           # 8.2 MB
    s_all = ph2.tile([128, MT_, 2], f32)
    nc.vector.memset(s_all, 0.0)

    ph2w = ctx.enter_context(tc.tile_pool(name="ph2w", bufs=2))
    ps2 = ctx.enter_context(tc.tile_pool(name="ps2", bufs=2, space="PSUM"))

    for mt in range(MT_):
        ms = slice(mt * MP_, (mt + 1) * MP_)
        for half in range(2):
            hs = slice(half * NHALF, (half + 1) * NHALF)
            lp = ps2.tile([MP_, 4, 512], f32, tag="logits")
            for nch in range(4):
                ns = slice(half * NHALF + nch * 500, half * NHALF + (nch + 1) * 500)
                nc.tensor.matmul(lp[:, nch, :500], ones_row[:, :MP_], bo_sb[:, ns],
                                 start=True, stop=False)
                for kc in range(KH):
                    nc.tensor.matmul(lp[:, nch, :500], aht_all[:, kc, ms],
                                     w_ot_sb[:, kc, ns], start=False,
                                     stop=(kc == KH - 1))
            lpv = lp[:, :, :500]
            # exp + partial sum (accum); fp32 exp kept for argmax
            et = ph2w.tile([MP_, 4, 500], f32, tag="exp")
            nc.scalar.activation(out=et, in_=lpv, func=AF.Exp,
                                 accum_out=s_all[:MP_, mt, half:half + 1])
            # bf16 raw logits for the normalize pass
            nc.vector.tensor_copy(
                out=logits_sb[:MP_, mt, hs].rearrange("p (c n) -> p c n", c=4),
                in_=lpv)
            # argmax candidates in exp domain
            mv = ph2w.tile([MP_, 8], f32, tag="mv")
            mi = ph2w.tile([MP_, 8], u32, tag="mi")
            nc.vector.max_with_indices(mv, mi, et.rearrange("p c n -> p (c n)"))
            nc.sync.dma_start(out=maxv[mt, half], in_=mv)
            nc.sync.dma_start(out=maxi[mt, half], in_=mi)

    # ---- AllReduce partial sums -> global logZ ----
    s_mt = ph2.tile([128, MT_], f32)
    nc.vector.tensor_add(out=s_mt, in0=s_all[:, :, 0], in1=s_all[:, :, 1])
    nc.sync.dma_start(out=cc_in.ap(), in_=s_mt)
    nc.gpsimd.collective_compute(
        "AllReduce", mybir.AluOpType.add,
        replica_groups=[list(range(NCORES))],
        ins=[cc_in.ap().opt()],
        outs=[cc_out.ap().opt()],
    )
    s_tot = ph2.tile([128, MT_], f32)
    nc.sync.dma_start(out=s_tot, in_=cc_out.ap())
    nlz = ph2.tile([128, MT_], f32)      # -log(sum_exp)
    nc.scalar.activation(out=nlz[:MP_], in_=s_tot[:MP_], func=AF.Ln)
    nc.vector.tensor_scalar_mul(out=nlz[:MP_], in0=nlz[:MP_], scalar1=-1.0)

    # ---- normalize + write out ----
    for mt in range(MT_):
        ms = slice(mt * MP_, (mt + 1) * MP_)
        for half in range(2):
            hs = slice(half * NHALF, (half + 1) * NHALF)
            ot = ph2w.tile([MP_, NHALF], f32, tag="out")
            nc.scalar.activation(out=ot, in_=logits_sb[:MP_, mt, hs],
                                 func=AF.Identity,
                                 bias=nlz[:MP_, mt:mt + 1], scale=1.0)
            nc.sync.dma_start(out=logp[ms, hs], in_=ot)


_CACHE = {}


def _get_program(tt=TT, detect_races=True):
    key = ("nc", tt)
    if key in _CACHE:
        return _CACHE[key]
    import concourse.tile as tile
    from concourse import bacc

    nc = bacc.Bacc(None, target_bir_lowering=False, num_devices=NCORES,
                   detect_race_conditions=detect_races)
    with tile.TileContext(nc) as tc:
        with ExitStack() as ctx:
            _build(nc, tc, ctx, tt=tt)
    if not nc.is_finalized():
        nc.finalize()
    _CACHE[key] = nc
    return nc


def _host_prep(inputs, tt=TT):
    """Layout marshalling only: casts, transposes, reshapes, gather, shard."""
    gi = lambda k: np.asarray(inputs[k])
    f = lambda k: np.asarray(inputs[k], dtype=np.float32)

    tgt = np.asarray(gi("tgt_batch"), dtype=np.int64)        # [B, T]
    emb = f("emb")                                           # [V, E]
    # step t uses word tgt[b, t]; row order r = t*B + b
    idx = tgt[:, :tt].T.reshape(-1)                          # [tt*B]
    X = emb[idx]                                             # [R, E] gather
    xt = np.ascontiguousarray(X.T)                           # [E, R]

    W_ih = f("W_ih")                                         # [4H, E+H]
    w_xt = np.ascontiguousarray(W_ih[:, :E].T)               # [E, 4H]
    w_aht = W_ih[:, E:].T                                    # [H, 4H]
    w_hht = f("W_hh").T                                      # [H, 4H]
    w_combt = np.ascontiguousarray(np.concatenate([w_aht, w_hht], axis=0))
    w_ct = np.ascontiguousarray(f("W_c").T)                  # [2H, H]
    w_at = np.ascontiguousarray(f("W_a").T)                  # [H, H]

    enc = f("enc_outputs")                                   # [S, B, H]
    enc_bsh = np.ascontiguousarray(enc.transpose(1, 0, 2))   # [B, S, H]
    enc_flat = enc_bsh.reshape(B * S, H)
    enct = np.ascontiguousarray(enc_flat.T)                  # [H, B*S]

    w_ot_full = np.ascontiguousarray(f("W_o").T)             # [H, V]
    b_o_full = f("b_o")

    common = dict(
        xt=xt, w_xt=w_xt, w_combt=w_combt, w_ct=w_ct, w_at=w_at,
        enc_flat=np.ascontiguousarray(enc_flat), enct=enct,
        b_ih=f("b_ih").reshape(1, -1), b_hh=f("b_hh").reshape(1, -1),
        b_c=f("b_c").reshape(1, -1),
        len_f=np.asarray(gi("len_src_batch"), dtype=np.float32).reshape(B, 1),
        h0=np.ascontiguousarray(f("h0")[0]),
        c0=np.ascontiguousarray(f("c0")[0]),
    )
    in_maps = []
    for c in range(NCORES):
        m = dict(common)
        m["w_ot"] = np.ascontiguousarray(w_ot_full[:, c * VS:(c + 1) * VS])
        m["b_o"] = np.ascontiguousarray(b_o_full[c * VS:(c + 1) * VS]).reshape(1, -1)
        in_maps.append(m)
    return in_maps


def _assemble(results, tt=TT):
    """Unshard: concat log-prob shards, combine argmax candidates."""
    R_ = tt * B
    logp_full = np.concatenate([r["logp"] for r in results], axis=1)
    dec_outputs = logp_full.reshape(tt, B, V)

    # candidates: value (exp domain) + global vocab index
    mt_ = max(1, R_ // MP) if R_ % MP == 0 else (R_ + MP - 1) // MP
    mp_ = R_ // mt_
    vals, idxs = [], []
    for c, r in enumerate(results):
        mv = r["maxv"].reshape(mt_, 2, mp_, 8)
        mi = r["maxi"].reshape(mt_, 2, mp_, 8).astype(np.int64)
        gi = c * VS + mi + np.arange(2).reshape(1, 2, 1, 1) * NHALF
        vals.append(mv.transpose(0, 2, 1, 3).reshape(R_, 16))
        idxs.append(gi.transpose(0, 2, 1, 3).reshape(R_, 16))
    av = np.concatenate(vals, axis=1)
    ai = np.concatenate(idxs, axis=1)
    # max value; ties -> smallest vocab index (matches argmax-first semantics)
    best = av.max(axis=1, keepdims=True)
    cand = np.where(av >= best, ai, np.int64(V))
    words = cand.min(axis=1).astype(np.int32).reshape(tt, B)
    return dec_outputs, words


def _run_hw(in_maps, trace=False, time_iters=0):
    import time as _time

    from concourse.bass_utils import run_bass_kernel_spmd

    nc = _get_program()
    res = run_bass_kernel_spmd(nc, in_maps, core_ids=list(range(NCORES)),
                               trace=trace)
    if time_iters:
        from concourse import bass2jax

        times = []
        for _ in range(time_iters):
            t0 = _time.perf_counter()
            bass2jax.run_bass_via_pjrt(nc, in_maps, n_cores=NCORES)
            times.append(_time.perf_counter() - t0)
        res.exec_time_ns = int(min(times) * 1e9)
    return res


def kernel(**inputs):
    in_maps = _host_prep(inputs)
    res = _run_hw(in_maps, trace=False)
    return _assemble(res.results)
